# revision 6
# baseline (speedup 1.0000x reference)
"""Trainium2 Bass kernel v3 for nn_SubspaceLinopFactory (subspace NUDFT).

v3 over v2: single merged phase-matmul per k-block (4 spans), batched input
DMAs (2 tensors + broadcast-DMA'd dcf), fp16 output, outputs via gpsimd DGE.

See kernel_v2.py docstring for the math. Table t5 spans: [sx|cx|sy|cy|nsy].
z spans = [-Im | Re] fp16; host gathers rows 4t+c for r = subsamp_idx[t].
"""
import numpy as np

A, T, C, R, D, K, H, W = 3, 32, 4, 8, 2, 1024, 64, 64
N_CORES = 8
KB = 256
NB = K // KB
MAGIC = float(1.5 * 2 ** 23)
TWO_PI = float(2 * np.pi)
RMB = 8     # merged m-build rows: 2 ty, 2 tx, flagx, 2 ty(y-side), flagy

_CACHE = {}


def _register_frac_op():
    import concourse.dve_ops as dops
    from concourse.dve_spec import Spec, Src0, C0, lower, _has_src1
    from concourse.dve_uop import DveOpSpec

    if "FRAC_ANT" in dops._SUB_OPCODE_FOR_NAME:
        return next(op for op in dops.OPS if op.name == "FRAC_ANT")

    spec = Spec(
        body=Src0 - ((Src0 + C0) - C0),
        reference=lambda in0, in1, s0, s1, imm2: (
            in0.astype(np.float32)
            - ((in0.astype(np.float32) + np.float32(s0)) - np.float32(s0))
        ).astype(np.float32),
    )
    opcode = max(dops._SUB_OPCODE_FOR_NAME.values()) + 1
    shas = {}
    for ver in ("v3", "v4"):
        s = DveOpSpec(name="FRAC_ANT", opcode=opcode, uops=lower(spec, ver=ver),
                      rd1_en=_has_src1(spec))
        shas[ver] = s.sha(ver)
    op = dops.DveOp("FRAC_ANT", spec, subdim=False, uops_sha=shas)
    dops.OPS.append(op)
    dops.CUSTOM_DVE_SPECS["FRAC_ANT"] = spec
    dops._SUB_OPCODE_FOR_NAME["FRAC_ANT"] = opcode
    return op


def _build_nc():
    import concourse.bacc as bacc
    import concourse.tile as tile
    import concourse.mybir as mybir
    from concourse.mybir import VecI64Pair

    AF = mybir.ActivationFunctionType
    OP = mybir.AluOpType
    F32 = mybir.dt.float32
    F16 = mybir.dt.float16
    BF16 = mybir.dt.bfloat16

    frac_op = _register_frac_op()
    nc = bacc.Bacc(None, target_bir_lowering=False)

    # bmb rows 0-6; cols: [cfx 128 | cfy 128 | rfx 2K | rfy 2K] bf16
    d_bmb = nc.dram_tensor("bmb", [5, 256 + 4 * K], BF16, kind="ExternalInput")
    # bfp: [128, 384 (sT) | 384 (wph)] f16
    d_bfp = nc.dram_tensor("bfp", [128, 2 * A * 128], F16, kind="ExternalInput")
    d_dcf = nc.dram_tensor("dcf", [1, 2, K], F16, kind="ExternalInput")
    d_z = nc.dram_tensor("z", [128, 2, K], F16, kind="ExternalOutput")

    with tile.TileContext(nc) as tc:
        with (
            tc.tile_pool(name="cst", bufs=1) as cst,
            tc.tile_pool(name="tbl", bufs=1) as tbl,
            tc.tile_pool(name="fr", bufs=2) as frp,
            tc.tile_pool(name="pq", bufs=3) as pqp,
            tc.tile_pool(name="uvp", bufs=3) as uvp,
            tc.tile_pool(name="psM", bufs=2, space="PSUM") as psM,
            tc.tile_pool(name="psPQ", bufs=2, space="PSUM") as psPQ,
            tc.tile_pool(name="psZ", bufs=2, space="PSUM") as psZ,
        ):
            bmb = cst.tile([5, 256 + 4 * K], BF16)
            bfp = cst.tile([128, 2 * A * 128], F16)
            dcf = cst.tile([128, 2, K], F16)
            nc.sync.dma_start(bmb[:], d_bmb[:])
            nc.sync.dma_start(bfp[:], d_bfp[:])
            nc.sync.dma_start(dcf[:], d_dcf[0:1].partition_broadcast(128))

            cfx = bmb[0:5, 0:128]
            cfy = bmb[0:3, 128:256]
            rfx = bmb[0:5, 256:256 + 2 * K].rearrange("p (s k) -> p s k", k=K)
            rfy = bmb[0:3, 256 + 2 * K:].rearrange("p (s k) -> p s k", k=K)
            sT = bfp[:, 0:A * 128].rearrange("p (a m) -> p a m", m=128)
            wph = bfp[:, A * 128:].rearrange("p (a m) -> p a m", m=128)

            t5 = tbl.tile([128, 5, K], F16)     # [sx|cx|sy|cy|nsy]
            zout = tbl.tile([128, 2, K], F16)

            def trig(b):
                ks = slice(b * KB, (b + 1) * KB)
                m = psM.tile([128, 4, KB], F32, tag="m")
                nc.tensor.matmul(m[:, 0:2, :], cfx[:], rfx[:, :, ks],
                                 start=True, stop=True)
                nc.tensor.matmul(m[:, 2:4, :], cfy[:], rfy[:, :, ks],
                                 start=True, stop=True)
                fr = frp.tile([128, 4, KB], F32, tag="fr")
                nc.vector._custom_dve(frac_op, out=fr[:], in0=m[:], s0=MAGIC)
                nc.scalar.activation(t5[:, 0:4, ks], fr[:], AF.Sin,
                                     scale=TWO_PI)

            def neg(b):
                ks = slice(b * KB, (b + 1) * KB)
                nc.scalar.activation(t5[:, 4, ks], t5[:, 2, ks], AF.Copy,
                                     scale=-1.0)

            def main(b):
                ks = slice(b * KB, (b + 1) * KB)
                z = psZ.tile([128, 2, KB], F32, tag="z")
                for a in range(A):
                    pq = psPQ.tile([128, 2, KB], F32, tag="pq")
                    nc.tensor.matmul(pq[:], sT[:, a, :], t5[:, 0:2, ks],
                                     start=True, stop=True)
                    pq16 = pqp.tile([128, 2, KB], F16, tag="pq16")
                    nc.scalar.copy(pq16[:], pq[:])
                    uv = uvp.tile([128, 4, KB], F16, tag="uv")
                    in0 = pq16[:].unsqueeze(2).broadcast_to([128, 2, 2, KB])
                    # in1 spans over t5: [cy(3), nsy(4), sy(2), cy(3)]
                    in1 = t5[:, 3, ks].unsqueeze(1).unsqueeze(1).broadcast_to(
                        [128, 2, 2, KB]).copy()
                    in1.ap = VecI64Pair(
                        [tuple(in1.ap[0]), (-K, 2), (K, 2), (1, KB)])
                    uvv = uv[:].rearrange("p (s d) k -> p s d k", s=2)
                    nc.vector.tensor_tensor(uvv[:], in0[:], in1, OP.mult)
                    if a == 0 and b + 1 < NB:
                        trig(b + 1)
                    nc.tensor.matmul(z[:], wph[:, a, :], uv[:, 0:2, :],
                                     start=(a == 0), stop=False,
                                     skip_group_check=True)
                    nc.tensor.matmul(z[:], wph[:, a, :], uv[:, 2:4, :],
                                     start=False, stop=(a == A - 1),
                                     skip_group_check=True)
                # dcf applied at the end: zout = z * dcf (both spans)
                nc.vector.tensor_tensor(zout[:, :, ks], z[:], dcf[:, :, ks],
                                        OP.mult)
                eng = nc.sync if b == NB - 1 else nc.gpsimd
                eng.dma_start(d_z[:, :, ks], zout[:, :, ks])

            trig(0)
            neg(0)
            for b in range(NB):
                main(b)
                if b + 1 < NB:
                    neg(b + 1)

    nc.finalize()
    return nc


def _split2(v):
    import ml_dtypes
    bf = ml_dtypes.bfloat16
    h = v.astype(bf)
    m = (v - h.astype(np.float64)).astype(bf)
    return h, m


def _stage_inputs(x, trj, phi, mps, sqrt_dcf):
    import ml_dtypes
    bf = ml_dtypes.bfloat16
    f16 = np.float16

    s = np.einsum("ahw,chw->achw", x.astype(np.float64), mps.astype(np.float64))
    s = s.reshape(A, C, 2, 32, W)
    sT = np.ascontiguousarray(
        s.transpose(2, 4, 0, 1, 3).reshape(2 * W, A, C * 32)).astype(f16)

    wph = np.zeros((128, A, 128), f16)
    for a in range(A):
        for c in range(C):
            wph[c * 32:(c + 1) * 32, a, np.arange(T) * 4 + c] = \
                phi[a].astype(f16)[None, :]

    bfp = np.concatenate([sT.reshape(128, -1), wph.reshape(128, -1)], axis=1)

    gx = np.arange(W, dtype=np.float64) - 32.0
    cfx = np.zeros((5, 128), np.float64)
    cfx[0:2] = np.repeat([0.0, 32.0], W)[None, :]
    cfx[2:4] = np.tile(gx, 2)[None, :]
    cfx[4] = 1.0
    cfy = np.zeros((5, 128), np.float64)
    cfy[0:2] = np.tile(np.arange(32, dtype=np.float64) - 32.0, 4)[None, :]
    cfy[2] = 1.0

    in_maps = []
    for r in range(N_CORES):
        tys = _split2(trj[r, 0].astype(np.float64) / (2 * np.pi))
        txs = _split2(trj[r, 1].astype(np.float64) / (2 * np.pi))
        rfx = np.zeros((5, 2, K), np.float64)
        rfy = np.zeros((5, 2, K), np.float64)
        for i in range(2):
            rfx[i, 0] = rfx[i, 1] = tys[i].astype(np.float64)
            rfx[2 + i, 0] = rfx[2 + i, 1] = txs[i].astype(np.float64)
            rfy[i, 0] = rfy[i, 1] = tys[i].astype(np.float64)
        rfx[4, 1] = 0.25
        rfy[2, 1] = 0.25
        bmb = np.concatenate(
            [cfx, cfy, rfx.reshape(5, -1), rfy.reshape(5, -1)],
            axis=1).astype(bf)
        in_maps.append({
            "bmb": bmb, "bfp": bfp,
            "dcf": np.ascontiguousarray(
                np.broadcast_to(sqrt_dcf[r].astype(f16)[None, None, :],
                                (1, 2, K))),
        })
    return in_maps


def kernel(x, trj, phi, mps, sqrt_dcf, subsamp_idx, _trace=False):
    from concourse.bass_utils import run_bass_kernel_spmd

    if "nc" not in _CACHE:
        _CACHE["nc"] = _build_nc()
    nc = _CACHE["nc"]
    in_maps = _stage_inputs(np.asarray(x), np.asarray(trj), np.asarray(phi),
                            np.asarray(mps), np.asarray(sqrt_dcf))
    res = run_bass_kernel_spmd(nc, in_maps, core_ids=list(range(N_CORES)),
                               trace=_trace)
    out = np.empty((T, C, K), dtype=np.complex64)
    idx = np.asarray(subsamp_idx).astype(np.int64)
    for t in range(T):
        z = res.results[int(idx[t])]["z"].astype(np.float32)
        rows = z[t * 4: t * 4 + 4]
        out[t, :, :] = rows[:, 1, :] - 1j * rows[:, 0, :]
    if _trace:
        kernel._last_results = res
    return out
